# revision 5
# baseline (speedup 1.0000x reference)
"""PocStrengthNet TRN2 kernel: tiny-MLP + per-segment softmax-weighted mean.

Self-contained: hardcodes shapes N=4194304, D=64, H=32, S=16384, 8 cores.

Math per row i: h = relu(x W1 + b1); r = h wr + br; z = h wz + bz;
per segment s: out[s] = SCALE * sum(exp(z) r') / sum(exp(z)) (softmax-weighted
mean; exp(z - max) in the reference cancels in the ratio, and z is O(1) so
plain exp is safe).

Device layout (per core, rows-on-partitions):
  - rows are fed in a permuted order: device row (tile t, partition p) is
    original row p*4096 + t, so each SBUF lane p owns a contiguous run of
    4096 sorted-segment rows.
  - mm1 (x-as-weights, fp16 FWL): lhsT = [xh; xl*2^6] fp16 [128,128] tile,
    rhs = [W2; W2*2^-6] fp16 [128, 64] -> psum [128 rows, 64] = x @ W2,
    where W2 = W1 * diag(|w|) with columns [z-pos, z-neg, r-pos, r-neg]
    (sign-split so relu(h+b1)*w becomes max(hz, -b1|w|) with +/- group sums).
  - vector: m = max(psum, thr); group reduces -> zP,zN,rP,rN columns;
    z = zP-zN (+const in exp bias), r = rP-rN; ez = exp(z+bz');
    u = (r+br')*ez; per-lane masked segment sums over 512-col chunks.
  - host: scatter-add per-(core,lane,chunk,slot) partial denom/numer into
    the S=16384 segment table; preds = numer/denom or DEFAULT.
"""
import os
import sys
import types

import numpy as np

# ---- environment shim (NTFF profile hook may be absent in the image) ----
if "antenv.axon_hooks" not in sys.modules:
    try:
        _m = types.ModuleType("antenv.axon_hooks")
        _m._hook = None

        def _set(h):
            _m._hook = h

        def _get():
            return _m._hook

        _m.set_axon_ntff_profile_hook = _set
        _m.get_axon_ntff_profile_hook = _get
        sys.modules["antenv.axon_hooks"] = _m
        import antenv

        antenv.axon_hooks = _m
        try:
            from trn_agent_boot.trn_boot import _ntff_profile_via_ctypes

            _set(_ntff_profile_via_ctypes("/opt/axon/libaxon_pjrt.so"))
        except Exception:
            pass
    except Exception:
        pass

import concourse.bacc as bacc
import concourse.mybir as mybir
from concourse import tile
import concourse.bass_utils as bass_utils
from concourse.bass_utils import run_bass_kernel_spmd

try:
    from concourse._compat import FishPath  # noqa: F401
    bass_utils.upload_artifacts = lambda tmpdir: tmpdir
except Exception:
    bass_utils.upload_artifacts = lambda tmpdir: tmpdir

f32 = mybir.dt.float32
f16 = mybir.dt.float16
AF = mybir.ActivationFunctionType
ALU = mybir.AluOpType
AX = mybir.AxisListType

SCALE = 400.0 / np.log(10.0)
DEFAULT_PRED = 7.6699353278706015

N, D, H, S = 4194304, 64, 32, 16384
N_CORES = 8
NC_ROWS = N // N_CORES            # 524288
LANES = 128
LANE_ROWS = NC_ROWS // LANES      # 4096 rows per lane (contiguous, sorted)
TILES = NC_ROWS // 128            # 4096 tiles of 128 rows
BANK_TILES = 8                    # tiles per psum bank ([128, 512] = 8*64)
BANKS = TILES // BANK_TILES       # 512
GROUP_BANKS = 4                   # reduce granularity: 4 banks = 32 tiles
CHUNK_COLS = 512                  # segsum chunk: 512 cols = 512 rows/lane
N_CHUNKS = LANE_ROWS // CHUNK_COLS  # 8
XCHUNK_BANKS = 8                  # DMA chunk = 8 banks = [128, 8192] cols

LAST_RESULT = None


def _host_prep(x, W1, b1, wr, br, wz, bz, segment_ids):
    """Build per-core device inputs + merge metadata."""
    x = np.asarray(x, dtype=np.float32)
    W1 = np.asarray(W1, dtype=np.float32)
    b1 = np.asarray(b1, dtype=np.float32)
    wr = np.asarray(wr, dtype=np.float32)
    wz = np.asarray(wz, dtype=np.float32)
    ids = np.asarray(segment_ids).astype(np.int64)

    # --- sign-split folded weights ---------------------------------------
    # W2 columns: [z-pos | z-neg | r-pos | r-neg]; col c from head weight w_j:
    #   W2[:, c] = W1[:, j] * |w_j|;  thr[c] = -b1[j] * |w_j|
    # z = sum_P m - sum_N m + b1@wz  (m = max(hz, thr))
    perm_cols = []
    thr_vals = []
    group_lens = []
    for w in (wz, wr):
        pos = np.where(w >= 0)[0]
        neg = np.where(w < 0)[0]
        group_lens.extend([len(pos), len(neg)])
        for j in np.concatenate([pos, neg]):
            perm_cols.append(W1[:, j] * np.abs(w[j]))
            thr_vals.append(-b1[j] * np.abs(w[j]))
    W2 = np.stack(perm_cols, axis=1)          # [64, 64] ordered zP|zN|rP|rN
    thr = np.array(thr_vals, dtype=np.float32)  # [64]

    # bias b1 is injected into PSUM (ones-row fold), so relu(psum) already
    # includes the +b1*|w| shift; only the head biases remain.
    bz_c = float(bz)
    br_c = float(br)

    # fp16 pair of W2 (hi for xh rows, lo-scaled for xl rows).
    # Row 127 (last xl row) is repurposed: lhsT carries ones there and the
    # rhs carries b1' = b1*|w| so the bias lands in PSUM and relu threshold
    # becomes 0 (costs only the tiny xl[63] contribution, ~1e-5 relative).
    W2h = W2.astype(np.float16)
    W2lo = (W2 * (2.0 ** -6)).astype(np.float16)
    W2s = np.concatenate([W2h, W2lo], axis=0)  # [128, 64] fp16
    W2s[127, :] = (-thr).astype(np.float16)

    # --- per-core x / ids in the lane-major feed order -------------------
    xpairs, ids_dense, bases = [], [], []
    for c in range(N_CORES):
        xc = x[c * NC_ROWS:(c + 1) * NC_ROWS]            # [NC, 64]
        xh = xc.astype(np.float16)
        xl = ((xc - xh.astype(np.float32)) * (2.0 ** 6)).astype(np.float16)
        # feed col (128 t + p) <- original row (p * LANE_ROWS + t)
        ph = xh.reshape(LANES, LANE_ROWS, D).transpose(2, 1, 0)  # [64,T,128]
        pl = xl.reshape(LANES, LANE_ROWS, D).transpose(2, 1, 0)
        xpair = np.concatenate(
            [ph.reshape(D, NC_ROWS), pl.reshape(D, NC_ROWS)], axis=0)
        xpair[127, :] = np.float16(1.0)
        xpairs.append(np.ascontiguousarray(xpair))        # [128, NC] f16

        idc = ids[c * NC_ROWS:(c + 1) * NC_ROWS].reshape(LANES, LANE_ROWS)
        ids_dense.append(np.ascontiguousarray(idc.astype(np.float32)))
        bases.append(idc[:, ::CHUNK_COLS].copy())         # [128, N_CHUNKS]

    # slots needed: max id-span within any (lane, chunk) run
    span = 0
    for c in range(N_CORES):
        idc = ids[c * NC_ROWS:(c + 1) * NC_ROWS].reshape(
            LANES, N_CHUNKS, CHUNK_COLS)
        span = max(span, int((idc[:, :, -1] - idc[:, :, 0]).max()) + 1)
    n_slots = max(span, 2)

    return dict(xpairs=xpairs, ids_dense=ids_dense, bases=bases,
                W2s=W2s, group_lens=group_lens,
                bz_c=bz_c, br_c=br_c, n_slots=n_slots)


def _build_program(prep):
    nc = bacc.Bacc("TRN2", target_bir_lowering=False, debug=False,
                   num_devices=N_CORES)
    gl = prep["group_lens"]          # [pz, nz, pr, nr]
    n_slots = prep["n_slots"]
    out_cols = N_CHUNKS * n_slots

    xp_d = nc.dram_tensor("xpair", [128, NC_ROWS], f16,
                          kind="ExternalInput").ap()
    ids_d = nc.dram_tensor("idsd", [128, LANE_ROWS], f32,
                           kind="ExternalInput").ap()
    w2_d = nc.dram_tensor("w2s", [128, 64], f16, kind="ExternalInput").ap()
    den_d = nc.dram_tensor("den", [128, out_cols], f32,
                           kind="ExternalOutput").ap()
    num_d = nc.dram_tensor("num", [128, out_cols], f32,
                           kind="ExternalOutput").ap()

    GB = GROUP_BANKS
    with tile.TileContext(nc) as tc:
        with (
            tc.tile_pool(name="const", bufs=1) as cpool,
            tc.tile_pool(name="xin", bufs=3) as xpool,
            tc.tile_pool(name="mbuf", bufs=2) as mpool,
            tc.tile_pool(name="acc", bufs=1) as apool,
            tc.tile_pool(name="seg", bufs=2) as spool,
            tc.tile_pool(name="ps", bufs=8, space="PSUM") as ps,
        ):
            w2_t = cpool.tile([128, 64], f16)
            nc.sync.dma_start(w2_t[:], w2_d)
            zero_t = cpool.tile([128, 1], f32)
            nc.vector.memset(zero_t[:], 0.0)
            ids_t = cpool.tile([128, LANE_ROWS], f32)
            nc.sync.dma_start(ids_t[:], ids_d)
            brc = cpool.tile([128, 1], f32)
            nc.vector.memset(brc[:], prep["br_c"])
            bzc = cpool.tile([128, 1], f32)
            nc.vector.memset(bzc[:], prep["bz_c"])

            zP = apool.tile([128, CHUNK_COLS], f32)
            zN = apool.tile([128, CHUNK_COLS], f32)
            rP = apool.tile([128, CHUNK_COLS], f32)
            rN = apool.tile([128, CHUNK_COLS], f32)
            den_acc = apool.tile([128, out_cols], f32)
            num_acc = apool.tile([128, out_cols], f32)

            accs = [zP, zN, rP, rN]
            offs = [0, gl[0], 32, 32 + gl[2]]

            for chunk in range(N_CHUNKS):
                # ---- MLP + reduce phase for this chunk (64 banks) ----
                for xc in range(BANKS // N_CHUNKS // XCHUNK_BANKS):
                    xci = chunk * (BANKS // N_CHUNKS // XCHUNK_BANKS) + xc
                    x_t = xpool.tile([128, XCHUNK_BANKS * 1024], f16,
                                     name=f"x_{xci}", tag="x")
                    nc.sync.dma_start(
                        x_t[:], xp_d[:, xci * 8192:(xci + 1) * 8192])
                    for g in range(XCHUNK_BANKS // GB):
                        m_t = mpool.tile([128, GB * 512], f32,
                                         name=f"m_{xci}_{g}", tag="m")
                        for bb in range(GB):
                            bank = xci * XCHUNK_BANKS + g * GB + bb
                            p_t = ps.tile([128, 512], f32,
                                          name=f"p_{bank}", tag="p")
                            for t in range(BANK_TILES):
                                col0 = (g * GB + bb) * 1024 + t * 128
                                nc.tensor.matmul(
                                    p_t[:, t * 64:t * 64 + 64],
                                    x_t[:, col0:col0 + 128],
                                    w2_t[:],
                                    start=True, stop=True)
                            # m = relu(psum) (bias already in PSUM)
                            nc.scalar.activation(
                                m_t[:, bb * 512:(bb + 1) * 512],
                                p_t[:], AF.Relu, bias=zero_t[:])
                        # group reduces: 4 outputs x 32 tiles
                        col0 = xc * XCHUNK_BANKS * BANK_TILES + \
                            g * GB * BANK_TILES
                        m3 = m_t[:].rearrange("p (a b) -> p a b", b=64)
                        for k in range(4):
                            nc.vector.tensor_reduce(
                                accs[k][:, col0:col0 + GB * BANK_TILES],
                                m3[:, :, offs[k]:offs[k] + gl[k]],
                                axis=AX.X, op=ALU.add)

                # ---- segment phase for this chunk ----
                c0 = chunk * CHUNK_COLS
                zd = spool.tile([128, 512], f32, name=f"zd{chunk}", tag="zd")
                nc.vector.tensor_tensor(
                    zd[:], zP[:], zN[:], op=ALU.subtract)
                rd = spool.tile([128, 512], f32, name=f"rd{chunk}", tag="rd")
                nc.vector.tensor_tensor(
                    rd[:], rP[:], rN[:], op=ALU.subtract)
                ez = spool.tile([128, 512], f32, name=f"ez{chunk}", tag="ez")
                nc.scalar.activation(ez[:], zd[:], AF.Exp, bias=bzc[:])
                rb = spool.tile([128, 512], f32, name=f"rb{chunk}", tag="rb")
                nc.scalar.activation(rb[:], rd[:], AF.Identity, bias=brc[:])
                u = spool.tile([128, 512], f32, name=f"u{chunk}", tag="u")
                nc.vector.tensor_tensor(u[:], rb[:], ez[:], op=ALU.mult)

                idsl = ids_t[:, c0:c0 + 512]
                base = ids_t[:, c0:c0 + 1]
                for j in range(n_slots):
                    msk = spool.tile([128, 512], f32,
                                     name=f"mk{chunk}_{j}", tag="mk")
                    nc.vector.tensor_scalar(
                        msk[:], idsl, scalar1=base, scalar2=float(j),
                        op0=ALU.subtract, op1=ALU.is_equal)
                    col = chunk * n_slots + j
                    me = spool.tile([128, 512], f32,
                                    name=f"me{chunk}_{j}", tag="me")
                    nc.vector.tensor_tensor(me[:], ez[:], msk[:],
                                            op=ALU.mult)
                    nc.vector.tensor_reduce(
                        den_acc[:, col:col + 1], me[:], axis=AX.X,
                        op=ALU.add)
                    mu = spool.tile([128, 512], f32,
                                    name=f"mu{chunk}_{j}", tag="mu")
                    nc.vector.tensor_tensor(mu[:], u[:], msk[:],
                                            op=ALU.mult)
                    nc.vector.tensor_reduce(
                        num_acc[:, col:col + 1], mu[:], axis=AX.X,
                        op=ALU.add)

            nc.sync.dma_start(den_d, den_acc[:])
            nc.sync.dma_start(num_d, num_acc[:])

    nc.compile()
    return nc


def kernel(x, W1, b1, wr, br, wz, bz, segment_ids, num_segments):
    global LAST_RESULT
    x = np.asarray(x)
    ids = np.asarray(segment_ids)
    assert x.shape == (N, D) and ids.shape == (N,)
    assert int(num_segments) == S

    prep = _host_prep(x, W1, b1, wr, br, wz, bz, ids)
    nc = _build_program(prep)

    in_maps = []
    for c in range(N_CORES):
        in_maps.append({
            "xpair": prep["xpairs"][c],
            "idsd": prep["ids_dense"][c],
            "w2s": prep["W2s"],
        })
    res = run_bass_kernel_spmd(nc, in_maps, list(range(N_CORES)),
                               trace=bool(os.environ.get("KERNEL_TRACE")))
    LAST_RESULT = res

    n_slots = prep["n_slots"]
    denom = np.zeros(S, dtype=np.float64)
    numer = np.zeros(S, dtype=np.float64)
    for c in range(N_CORES):
        den = res.results[c]["den"].astype(np.float64)  # [128, chunks*slots]
        num = res.results[c]["num"].astype(np.float64)
        base = prep["bases"][c]                         # [128, N_CHUNKS]
        segs = np.minimum(
            base[:, :, None] + np.arange(n_slots)[None, None, :],
            S - 1).reshape(-1)
        np.add.at(denom, segs.astype(np.int64),
                  den.reshape(128, N_CHUNKS, n_slots).reshape(-1))
        np.add.at(numer, segs.astype(np.int64),
                  num.reshape(128, N_CHUNKS, n_slots).reshape(-1))

    nonempty = denom > 0
    preds = np.where(nonempty, numer / np.where(nonempty, denom, 1.0),
                     DEFAULT_PRED)
    return (SCALE * preds).astype(np.float32)
